# revision 7
# baseline (speedup 1.0000x reference)
"""Block-diagonal projection kernel for Trainium2 (8 NeuronCores, SPMD).

Math: out[b,s,h,o] = sum_i inputs[b,s,h,i] * W[h,o,i]
Shapes: inputs [8, 2048, 16, 128] f32, W [16, 128, 128] f32.

Sharding: data-parallel over batch — core b handles inputs[b] (no
communication).

Precision: all HBM traffic is bf16 (inputs, W, output); the matmul
accumulates in fp32 PSUM. bf16 rounding contributes ~0.5% relative
error, far under the 2e-2 gate, and halves the HBM traffic that
bounds this kernel (fp32: 33 MiB/core ≈ 96 us at 358 GB/s; bf16:
16.5 MiB ≈ 48 us).

Host-side layout prep puts the contraction dim (i) on SBUF partitions
so the device kernel is pure matmul streaming, and pre-chunks the s
axis so every input DMA reads 8 KB-contiguous per-partition lines:
  x per core: [c, i=128, h=16, sc]  (from inputs[b] [s,h,i], s = c*SC+sc)
  w (shared): [i=128, h=16, o=128]  (W.transpose(2,0,1))
Per 128-row s-tile and head h:
  psum[s128, o] = lhsT.T @ rhs, lhsT = x[c][:, h, s128] (stationary,
  [i,128]), rhs = w[:, h, :] ([i, o=128]).  Output lands in natural
[s, h, o] layout, so stores need no transposition anywhere on device.

Schedule: the kernel is HBM/fabric-bandwidth-bound (~17 MB at ~420
GB/s/core dual-queue), so the goal is one gapless DMA stream with BOTH
HWDGE queues loaded the whole time (a single queue only sustains ~300
GB/s). Inputs are front-loaded on both rings (SP: chunk-0 quarters +
chunks 5-7; ACT: w + chunks 1-4, all issued unconditionally at kernel
start), and the output tiles are split likewise (SP: tiles 0-7 +
last-tile quarters 0,1; ACT: tiles 8-14 + quarters 2,3), gated only by
copy completion. By the time the rings finish the input stream, many
output tiles are already copied, so the SDMA engines never idle until
the output backlog drains; the final tile's matmul->copy->DMA chain
completes long before the stream ends, eliminating the tail bubble.

PSUM->SBUF cast copies (fp32 -> bf16) are split between DVE
(head-groups 0,1) and ACT (head-groups 2,3) with separate completion
sems (s_cpv / s_cpa) so per-tile completion can be tested without
relying on cross-engine ordering.

Raw-bass engine programs (not Tile): walrus's PE instruction structs
accept at most one sync-wait per instruction, so all cross-engine sync
is standalone wait_ge instructions + then_inc updates:
  SP   : x chunk-0 quarter DMAs, all output DMAs
  ACT  : w DMA, x chunk 1..7 DMAs, then half the PSUM->SBUF copies
  PE   : 4 matmuls per (s-tile, head-group) into one PSUM bank
  DVE  : the other half of the PSUM->SBUF copies
All 8 input chunks stay resident in SBUF (bf16 halves the footprint),
so input buffers are never recycled and input DMAs need no waits.
"""

from contextlib import ExitStack

import ml_dtypes
import numpy as np

import concourse.bass as bass
import concourse.mybir as mybir
from concourse.bass_utils import run_bass_kernel_spmd

F32 = mybir.dt.float32
BF16 = mybir.dt.bfloat16
NP_BF16 = ml_dtypes.bfloat16

B, S, H, NI, NO = 8, 2048, 16, 128, 128
N_CORES = 8
SC = 256  # s rows per input chunk (H*NI*SC*2 = 1 MiB per chunk DMA)
CH = S // SC  # 8 chunks
XBUFS = CH  # all input chunks resident in SBUF (8 x 8 KB/partition)
OBUFS = 8  # out-tile SBUF buffers (8 x 4 KB/partition)
NBANKS = 8  # PSUM banks used (one head-group of 4 matmuls per bank)


def build_nc(s=S, h=H, ni=NI, no=NO, sc=SC):
    assert s % sc == 0 and sc % 128 == 0 and h % 4 == 0
    nt = s // 128  # 128-row s-tiles
    gpt = h // 4  # head-groups per s-tile
    ng = nt * gpt  # total matmul groups
    gpc = (sc // 128) * gpt  # groups per chunk
    ch = s // sc  # chunks
    tpc = sc // 128  # tiles per chunk

    nc = bass.Bass()
    x = nc.dram_tensor("x", [ch, ni, h, sc], BF16, kind="ExternalInput")
    w = nc.dram_tensor("w", [ni, h, no], BF16, kind="ExternalInput")
    y = nc.dram_tensor("y", [s, h, no], BF16, kind="ExternalOutput")

    ctx = ExitStack()
    with ctx:
        xts = [ctx.enter_context(nc.sbuf_tensor(f"xt{i}", [ni, h, sc], BF16)) for i in range(XBUFS)]
        ots = [ctx.enter_context(nc.sbuf_tensor(f"ot{i}", [128, h, no], BF16)) for i in range(OBUFS)]
        wt = ctx.enter_context(nc.sbuf_tensor("wt", [ni, h, no], BF16))
        pss = [ctx.enter_context(nc.psum_tensor(f"ps{i}", [128, 4, no], F32)) for i in range(NBANKS)]
        # Per-chunk / per-buffer-slot DMA-completion sems: two in-flight
        # DMAs incrementing one sem can interleave their 16 per-engine
        # increments, so a shared counter would not say WHICH transfer
        # finished.
        s_x = [ctx.enter_context(nc.semaphore(f"s_x{i}")) for i in range(ch)]
        s_yd = [ctx.enter_context(nc.semaphore(f"s_yd{i}")) for i in range(OBUFS)]
        # chunk 0 and w are split into per-head-group quarter DMAs so the
        # first matmuls start as soon as their slice lands.
        s_x0q = [ctx.enter_context(nc.semaphore(f"s_x0q{i}")) for i in range(gpt)]
        s_wq = [ctx.enter_context(nc.semaphore(f"s_wq{i}")) for i in range(gpt)]
        s_pe = ctx.enter_context(nc.semaphore("s_pe"))
        s_cpv = ctx.enter_context(nc.semaphore("s_cpv"))  # DVE copies (gg 0,1)
        s_cpa = ctx.enter_context(nc.semaphore("s_cpa"))  # ACT copies (gg 2,3)
        block = ctx.enter_context(nc.Block())

        # how many s_yd[] increments each slot sees in total (full tiles
        # inc 16; the last tile goes out as 4 quarter-DMAs, 16 each)
        slot_total = [0] * OBUFS
        for t2 in range(nt - 1):
            slot_total[t2 % OBUFS] += 16
        slot_total[(nt - 1) % OBUFS] += 16 * gpt

        # input chunks per ring (chunk 0 goes out quartered on SP)
        ACT_CHUNKS = [c for c in range(1, ch) if c <= ch // 2]
        SP_CHUNKS = [c for c in range(1, ch) if c > ch // 2]
        # output tiles per ring (ACT issues a tile right after copying it)
        SP_TILES = list(range(0, nt // 2))
        ACT_TILES = list(range(nt // 2, nt - 1))

        @block.sync
        def _(sp):
            for q in range(gpt):
                sp.dma_start(
                    xts[0][:, 4 * q : 4 * (q + 1), :], x[0][:, 4 * q : 4 * (q + 1), :]
                ).then_inc(s_x0q[q], 16)
            for c in SP_CHUNKS:
                sp.dma_start(xts[c][:], x[c]).then_inc(s_x[c], 16)
            for t in SP_TILES:
                sp.wait_ge(s_cpv, 2 * (t + 1))
                sp.wait_ge(s_cpa, 2 * (t + 1))
                sp.dma_start(y[t * 128 : (t + 1) * 128, :, :], ots[t % OBUFS][:]).then_inc(
                    s_yd[t % OBUFS], 16
                )
            # last tile: quarter-DMAs 0,1, each gated on just the DVE copy
            # that produced it
            t = nt - 1
            for q in (0, 1):
                sp.wait_ge(s_cpv, 2 * t + q + 1)
                sp.dma_start(
                    y[t * 128 : (t + 1) * 128, 4 * q : 4 * (q + 1), :],
                    ots[t % OBUFS][:, 4 * q : 4 * (q + 1), :],
                ).then_inc(s_yd[t % OBUFS], 16)
            # data-landed waits for every output DMA (both rings; the sems
            # are global counters so each engine can wait the full totals)
            for sl in range(OBUFS):
                if slot_total[sl]:
                    sp.wait_ge(s_yd[sl], slot_total[sl])

        @block.tensor
        def _(pe):
            for g in range(ng):
                t = g // gpt  # s-tile index
                c = t // tpc  # chunk index
                # Waits are consolidated per TILE: every standalone wait_ge
                # drains the PE pipeline, so one pair of copy-done waits
                # covers all 4 banks of the tile (tile t reuses tile t-2's
                # banks).
                if t == 0:
                    pe.wait_ge(s_wq[g % gpt], 16)
                    pe.wait_ge(s_x0q[g % gpt], 16)
                elif g % gpt == 0:
                    if g % gpc == 0:
                        pe.wait_ge(s_x[c], 16)
                    if t >= 2:
                        pe.wait_ge(s_cpv, 2 * (t - 1))
                        pe.wait_ge(s_cpa, 2 * (t - 1))
                xt = xts[c]
                t_in_c = t - c * tpc
                ps = pss[g % NBANKS]
                for j in range(4):
                    hh = (g % gpt) * 4 + j
                    mm = pe.matmul(
                        ps[:, j, :],
                        xt[:, hh, t_in_c * 128 : (t_in_c + 1) * 128],
                        wt[:, hh, :],
                        start=(j == 0),
                        stop=(j == 3),
                    )
                mm.then_inc(s_pe, 1)

        @block.vector
        def _(dve):
            for t in range(nt):
                if t >= OBUFS:
                    dve.wait_ge(s_yd[t % OBUFS], 16 * (t // OBUFS))
                dve.wait_ge(s_pe, gpt * t + 2)
                for gg in (0, 1):
                    dve.tensor_copy(
                        ots[t % OBUFS][:, gg * 4 : (gg + 1) * 4, :],
                        pss[(gpt * t + gg) % NBANKS][:],
                    ).then_inc(s_cpv, 1)

        @block.scalar
        def _(act):
            for q in range(gpt):
                act.dma_start(
                    wt[:, 4 * q : 4 * (q + 1), :], w[:, 4 * q : 4 * (q + 1), :]
                ).then_inc(s_wq[q], 16)
            for c in ACT_CHUNKS:
                act.dma_start(xts[c][:], x[c]).then_inc(s_x[c], 16)
            for t in range(nt):
                if t >= OBUFS:
                    act.wait_ge(s_yd[t % OBUFS], 16 * (t // OBUFS))
                act.wait_ge(s_pe, gpt * t + 4)
                for gg in (2, 3):
                    act.copy(
                        ots[t % OBUFS][:, gg * 4 : (gg + 1) * 4, :],
                        pss[(gpt * t + gg) % NBANKS][:],
                    ).then_inc(s_cpa, 1)
                if t in ACT_TILES:
                    # own copies are done (s_cpa wait also fences their
                    # writeback); only DVE's half still needs a check
                    act.wait_ge(s_cpa, 2 * (t + 1))
                    act.wait_ge(s_cpv, 2 * (t + 1))
                    act.dma_start(
                        y[t * 128 : (t + 1) * 128, :, :], ots[t % OBUFS][:]
                    ).then_inc(s_yd[t % OBUFS], 16)
                elif t == nt - 1:
                    for q in (2, 3):
                        act.wait_ge(s_cpa, 2 * t + q - 1)
                        act.dma_start(
                            y[t * 128 : (t + 1) * 128, 4 * q : 4 * (q + 1), :],
                            ots[t % OBUFS][:, 4 * q : 4 * (q + 1), :],
                        ).then_inc(s_yd[t % OBUFS], 16)
            for sl in range(OBUFS):
                if slot_total[sl]:
                    act.wait_ge(s_yd[sl], slot_total[sl])

    return nc


_NC_CACHE = {}


def _get_nc():
    if "nc" not in _NC_CACHE:
        _NC_CACHE["nc"] = build_nc()
    return _NC_CACHE["nc"]


def run(inputs, W, trace=False):
    """Returns (out [B,S,H,NO] f32, BassKernelResults)."""
    import os

    if trace:
        os.environ.pop("BASS_NEVER_TRACE", None)
    else:
        # The axon NTFF profiling hook module isn't present in this image;
        # make sure a stray BASS_TRACE can't route us onto that path.
        os.environ.setdefault("BASS_NEVER_TRACE", "1")
    inputs = np.asarray(inputs, dtype=np.float32)
    W = np.asarray(W, dtype=np.float32)
    assert inputs.shape == (B, S, H, NI) and W.shape == (H, NO, NI)
    # [b, s, h, i] -> [b, c, sc, h, i] -> [b, c, i, h, sc], cast to bf16
    xh = np.ascontiguousarray(
        inputs.reshape(B, CH, SC, H, NI).transpose(0, 1, 4, 3, 2)
    ).astype(NP_BF16)
    wh = np.ascontiguousarray(W.transpose(2, 0, 1)).astype(NP_BF16)  # [i, h, o]
    in_maps = [{"x": xh[b], "w": wh} for b in range(N_CORES)]
    br = run_bass_kernel_spmd(_get_nc(), in_maps, list(range(N_CORES)), trace=trace)
    out = np.stack([r["y"] for r in br.results]).astype(np.float32)  # [b, s, h, o]
    return out, br


def kernel(inputs, W):
    out, _ = run(inputs, W)
    return out
